# revision 9
# baseline (speedup 1.0000x reference)
"""Multi-head attention (B=4, T=2048, D=1024, H=16, DH=64) on 8 Trainium2 NeuronCores.

Sharding (data + tensor parallel, no collectives):
  core c owns batch b = c//2 and heads [(c%2)*8, (c%2)*8 + 8).
  Each core computes q/k/v projections for its batch over its 8 heads, the
  full attention for those (batch, head) slabs, and a partial output
  projection over its heads' columns of w_out.  The host sums the two
  partial outputs per batch (the only cross-core reduction).

Device algorithm (per core), everything feature-major ("transposed") so the
contraction dim always lands on SBUF partitions:
  qT = Wq @ x^T            [512, T]   (rows = h_local*64 + d)
  kT = Wk @ x^T            [512, T]
  V  = x @ Wv^T            [T, 512]   (+ a ones column per head => denominator)
  per head h: ST = kT_h^T-contraction  => scores [j, i] in PSUM,
              E = exp(ST/32) (ScalarE, fused scale),
              oT_h' = [V_h | 1]^T @ E  => [65, i] (row 64 = softmax denom),
              oT_h = oT_h[0:64] * (1/denom) broadcast (GPSIMD bcast + DVE)
  out_partial = oT^T @ Wo_slice^T  [T, 1024]
All matmuls run in float32r (fp32 storage, full-rate PE mode, ~1e-5 rel err).
"""

import os
import sys

import numpy as np

if "/opt/trn_rl_repo" not in sys.path and os.path.isdir("/opt/trn_rl_repo"):
    sys.path.insert(0, "/opt/trn_rl_repo")

import concourse.tile as tile  # noqa: E402
from concourse import bacc, mybir  # noqa: E402
from concourse.bass_utils import run_bass_kernel_spmd  # noqa: E402

F32 = mybir.dt.float32
F32R = mybir.dt.float32r
AF = mybir.ActivationFunctionType
OP = mybir.AluOpType

B, T, D, H, DH = 4, 2048, 1024, 16, 64
HL = H // 2          # heads per core
E = HL * DH          # 512: per-core q (or k, or v) feature width
KO = D // 128        # 8 contraction chunks for the projections
P = 128
SCALE = float(D) ** -0.5  # note: dim**-0.5, faithful to the reference

_cache = {}
last_results = None


def _emit(ctx, tc, nc, xT, wq, wk, wv, wo, out, t, dbg=None):
    TB = t // 512        # moving-dim blocks for projections
    TC = t // 128        # t chunks (also j chunks)
    JC = t // 128
    IBN = t // 512       # query i-blocks
    GN = JC // 4         # groups of 4 j-chunks per i-block

    xT3 = xT.rearrange("(ko p) t -> p ko t", p=P)
    wq3 = wq.rearrange("(ko p) e -> p ko e", p=P)
    wk3 = wk.rearrange("(ko p) e -> p ko e", p=P)
    wv3 = wv.rearrange("(ko p) e -> p ko e", p=P)
    wo3 = wo.rearrange("(c p) d -> p c d", p=P)

    persist = ctx.enter_context(tc.tile_pool(name="persist", bufs=1))
    qkT = persist.tile([P, 8, t], F32R, tag="qkT")    # outer 0-3: q pairs, 4-7: k pairs
    vsb = persist.tile([P, JC, HL, DH + 1], F32R, tag="v")
    oT_all = persist.tile([P, 4, t], F32R, tag="oT")
    ones1 = persist.tile([P, 1], F32, tag="ones")
    nc.vector.memset(ones1[:], 1.0)
    nc.vector.tensor_copy(vsb[:, :, :, DH], ones1.to_broadcast((P, JC, HL)))

    # ---- phase 1a: V projection (x pass 1) ----
    with tc.tile_pool(name="wv_p", bufs=1) as wvp, \
         tc.tile_pool(name="xt1", bufs=2) as xtp1, \
         tc.tile_pool(name="ps1", bufs=4, space="PSUM") as ps1:
        wv_sb = wvp.tile([P, KO, E], F32R, tag="wv")
        nc.sync.dma_start(wv_sb[:], wv3)
        for tb in range(TB):
            xt = xtp1.tile([P, KO, 512], F32R, tag="xt")
            nc.sync.dma_start(xt[:], xT3[:, :, tb * 512:(tb + 1) * 512])
            for sub in range(4):
                tci = tb * 4 + sub
                ps = ps1.tile([P, 512], F32, tag="vps")
                for ko in range(KO):
                    nc.tensor.matmul(ps[:], xt[:, ko, sub * 128:(sub + 1) * 128],
                                     wv_sb[:, ko, :],
                                     start=(ko == 0), stop=(ko == KO - 1))
                nc.vector.tensor_copy(vsb[:, tci, :, 0:DH],
                                      ps.rearrange("p (h d) -> p h d", d=DH))

    # ---- phase 1b: Q and K projections (x pass 2) ----
    with tc.tile_pool(name="wqk_p", bufs=1) as wqkp, \
         tc.tile_pool(name="xt2", bufs=2) as xtp2, \
         tc.tile_pool(name="ps2", bufs=4, space="PSUM") as ps2:
        wq_sb = wqkp.tile([P, KO, E], F32R, tag="wq")
        wk_sb = wqkp.tile([P, KO, E], F32R, tag="wk")
        nc.sync.dma_start(wq_sb[:], wq3)
        nc.sync.dma_start(wk_sb[:], wk3)
        for tb in range(TB):
            xt = xtp2.tile([P, KO, 512], F32R, tag="xt")
            nc.sync.dma_start(xt[:], xT3[:, :, tb * 512:(tb + 1) * 512])
            for eo in range(8):
                wsb = wq_sb if eo < 4 else wk_sb
                ee = (eo % 4) * 128
                ps = ps2.tile([P, 512], F32, tag="qkps")
                for ko in range(KO):
                    nc.tensor.matmul(ps[:], wsb[:, ko, ee:ee + 128], xt[:, ko, :],
                                     start=(ko == 0), stop=(ko == KO - 1))
                nc.vector.tensor_copy(qkT[:, eo, tb * 512:(tb + 1) * 512], ps[:])

    # ---- phase 2: attention, head pairs (2p at partitions 0-63, 2p+1 at 64-127) ----
    with tc.tile_pool(name="attn", bufs=1) as asb, \
         tc.tile_pool(name="expp", bufs=2) as expp, \
         tc.tile_pool(name="bc", bufs=1) as bcp, \
         tc.tile_pool(name="qka", bufs=1, space="PSUM") as qka, \
         tc.tile_pool(name="qkb", bufs=1, space="PSUM") as qkb:
        for pr in range(4):
            qa = qkT[0:64, pr, :]
            ka = qkT[0:64, 4 + pr, :]
            qb = qkT[64:128, pr, :]
            kb = qkT[64:128, 4 + pr, :]
            oacc_a = asb.tile([65, t], F32, tag="oacc_a")
            oacc_b = asb.tile([65, t], F32, tag="oacc_b")
            for ib in range(IBN):
                i0 = ib * 512
                for g in range(GN):
                    psa = qka.tile([P, 4, 512], F32, tag="A")
                    psb = qkb.tile([P, 4, 512], F32, tag="B")
                    # scores^T for 4 j-chunks; the two heads run in the two
                    # row-halves of the PE array concurrently
                    for jj in range(4):
                        jc = g * 4 + jj
                        nc.tensor.matmul(psa[:, jj, :], ka[:, jc * 128:(jc + 1) * 128],
                                         qa[:, i0:i0 + 512], start=True, stop=True)
                        nc.tensor.matmul(psb[:, jj, :], kb[:, jc * 128:(jc + 1) * 128],
                                         qb[:, i0:i0 + 512], start=True, stop=True)
                    ea = expp.tile([P, 4, 512], F32R, tag="ea")
                    eb = expp.tile([P, 4, 512], F32R, tag="eb")
                    nc.scalar.activation(ea[:], psa[:], AF.Exp, scale=SCALE)
                    nc.scalar.activation(eb[:], psb[:], AF.Exp, scale=SCALE)
                    # P^T @ [V|1]: accumulate over the group's 4 j-chunks into
                    # bank 0 of the same psum tile (scores there already consumed)
                    ava = psa[0:65, 0, :]
                    avb = psb[0:65, 0, :]
                    for jj in range(4):
                        jc = g * 4 + jj
                        nc.tensor.matmul(ava, vsb[:, jc, 2 * pr, :], ea[:, jj, :],
                                         start=(jj == 0), stop=(jj == 3))
                    for jj in range(4):
                        jc = g * 4 + jj
                        nc.tensor.matmul(avb, vsb[:, jc, 2 * pr + 1, :], eb[:, jj, :],
                                         start=(jj == 0), stop=(jj == 3))
                    if g == 0:
                        nc.vector.tensor_copy(oacc_a[:, i0:i0 + 512], ava)
                        nc.vector.tensor_copy(oacc_b[:, i0:i0 + 512], avb)
                    else:
                        nc.vector.tensor_tensor(oacc_a[:, i0:i0 + 512],
                                                oacc_a[:, i0:i0 + 512], ava, OP.add)
                        nc.vector.tensor_tensor(oacc_b[:, i0:i0 + 512],
                                                oacc_b[:, i0:i0 + 512], avb, OP.add)
            # normalize: oT = oacc[0:64] * (1 / denom), denom = row 64.
            # The denom row sits at partition 64; engines can't shift
            # partitions, so DMA it to a partition-0 tile first.  The B half's
            # product must land at partitions 64-127 of oT_all — also a
            # partition shift, done with an SBUF->SBUF DMA of the product.
            rec = asb.tile([1, t], F32, tag="rec")
            nc.sync.dma_start(rec[:], oacc_a[64:65, :])
            nc.vector.reciprocal(rec[:], rec[:])
            ba = bcp.tile([64, t], F32, tag="bc")
            nc.gpsimd.partition_broadcast(ba[:], rec[:])
            nc.vector.tensor_tensor(oT_all[0:64, pr, :], oacc_a[0:64, :], ba[:], OP.mult)
            nc.sync.dma_start(rec[:], oacc_b[64:65, :])
            nc.vector.reciprocal(rec[:], rec[:])
            bb = bcp.tile([64, t], F32, tag="bc")
            nc.gpsimd.partition_broadcast(bb[:], rec[:])
            tmpb = bcp.tile([64, t], F32R, tag="tmpb")
            nc.vector.tensor_tensor(tmpb[:], oacc_b[0:64, :], bb[:], OP.mult)
            nc.sync.dma_start(oT_all[64:128, pr, :], tmpb[:])

    if dbg is not None:
        nc.sync.dma_start(dbg["qkT"], qkT[:].bitcast(F32))
        nc.sync.dma_start(dbg["vsb"], vsb[:].bitcast(F32))
        nc.sync.dma_start(dbg["oT"], oT_all[:].bitcast(F32))

    # ---- phase 3: output projection (partial over this core's heads) ----
    with tc.tile_pool(name="wo_p", bufs=1) as wop, \
         tc.tile_pool(name="op_ps", bufs=4, space="PSUM") as opps, \
         tc.tile_pool(name="osb", bufs=4) as osbp:
        wo_sb = wop.tile([P, 4, D], F32R, tag="wo")
        nc.sync.dma_start(wo_sb[:], wo3)
        for tci in range(TC):
            for db in range(D // 512):
                ps = opps.tile([P, 512], F32, tag="op")
                for c in range(4):
                    nc.tensor.matmul(ps[:], oT_all[:, c, tci * 128:(tci + 1) * 128],
                                     wo_sb[:, c, db * 512:(db + 1) * 512],
                                     start=(c == 0), stop=(c == 3))
                ot = osbp.tile([P, 512], F32, tag="ot")
                nc.vector.tensor_copy(ot[:], ps[:])
                nc.sync.dma_start(out[tci * 128:(tci + 1) * 128, db * 512:(db + 1) * 512],
                                  ot[:])


def _build(t, debug_outs=False):
    from contextlib import ExitStack

    nc = bacc.Bacc("TRN2", target_bir_lowering=False, debug=False, num_devices=8)
    xT = nc.dram_tensor("xT", [D, t], F32R, kind="ExternalInput").ap()
    wq = nc.dram_tensor("wq", [D, E], F32R, kind="ExternalInput").ap()
    wk = nc.dram_tensor("wk", [D, E], F32R, kind="ExternalInput").ap()
    wv = nc.dram_tensor("wv", [D, E], F32R, kind="ExternalInput").ap()
    wo = nc.dram_tensor("wo", [E, D], F32R, kind="ExternalInput").ap()
    out = nc.dram_tensor("out", [t, D], F32, kind="ExternalOutput").ap()
    dbg = None
    if debug_outs:
        JC = t // 128
        dbg = {
            "qkT": nc.dram_tensor("dbg_qkT", [P, 8, t], F32, kind="ExternalOutput").ap(),
            "vsb": nc.dram_tensor("dbg_vsb", [P, JC, HL, DH + 1], F32, kind="ExternalOutput").ap(),
            "oT": nc.dram_tensor("dbg_oT", [P, 4, t], F32, kind="ExternalOutput").ap(),
        }
    with tile.TileContext(nc) as tc:
        with ExitStack() as ctx:
            _emit(ctx, tc, nc, xT, wq, wk, wv, wo, out, t, dbg)
    nc.compile()
    return nc


def get_compiled(t=T, debug_outs=False):
    key = (t, debug_outs)
    if key not in _cache:
        _cache[key] = _build(t, debug_outs)
    return _cache[key]


def shard_inputs(x, w_qkv, t=T):
    """Per-core input maps (weights reordered head-major, x transposed)."""
    d_idx = np.arange(DH)
    maps = []
    for c in range(8):
        b = c // 2
        heads = np.arange((c % 2) * HL, (c % 2) * HL + HL)
        rows_q = (heads[:, None] + d_idx[None, :] * (3 * H)).reshape(-1)
        rows_k = (heads[:, None] + H + d_idx[None, :] * (3 * H)).reshape(-1)
        rows_v = (heads[:, None] + 2 * H + d_idx[None, :] * (3 * H)).reshape(-1)
        maps.append({
            "xT": np.ascontiguousarray(x[b][:t].T),
            "wq": np.ascontiguousarray(w_qkv[rows_q].T),
            "wk": np.ascontiguousarray(w_qkv[rows_k].T),
            "wv": np.ascontiguousarray(w_qkv[rows_v].T),
            "wo": None,  # filled below
        })
    return maps


def kernel(x, w_qkv, w_out, b_out):
    x = np.asarray(x, dtype=np.float32)
    w_qkv = np.asarray(w_qkv, dtype=np.float32)
    w_out = np.asarray(w_out, dtype=np.float32)
    b_out = np.asarray(b_out, dtype=np.float32)

    nc = get_compiled(T)
    d_idx = np.arange(DH)
    in_maps = shard_inputs(x, w_qkv, T)
    for c in range(8):
        heads = np.arange((c % 2) * HL, (c % 2) * HL + HL)
        cols_o = (heads[:, None] * DH + d_idx[None, :]).reshape(-1)
        in_maps[c]["wo"] = np.ascontiguousarray(w_out[:, cols_o].T)

    res = run_bass_kernel_spmd(nc, in_maps, core_ids=list(range(8)))
    global last_results
    last_results = res

    out = np.empty((B, T, D), dtype=np.float32)
    for b in range(B):
        out[b] = res.results[2 * b]["out"] + res.results[2 * b + 1]["out"]
    out += b_out
    return out.reshape(B, T, D)


# revision 11
# speedup vs baseline: 1.5391x; 1.5391x over previous
"""Multi-head attention (B=4, T=2048, D=1024, H=16, DH=64) on 8 Trainium2 NeuronCores.

Sharding (data + tensor parallel, no collectives):
  core c owns batch b = c//2 and heads [(c%2)*8, (c%2)*8 + 8).
  Each core computes q/k/v projections for its batch over its 8 heads, the
  full attention for those (batch, head) slabs, and a partial output
  projection over its heads' columns of w_out.  The host sums the two
  partial outputs per batch (the only cross-core reduction).

Device algorithm (per core), everything feature-major ("transposed") so the
contraction dim always lands on SBUF partitions:
  qT = Wq @ x^T            [512, T]   (rows = h_local*64 + d)
  kT = Wk @ x^T            [512, T]
  V  = x @ Wv^T            [T, 512]   (+ a ones column per head => denominator)
  per head h: ST = kT_h^T-contraction  => scores [j, i] in PSUM,
              E = exp(ST/32) (ScalarE, fused scale),
              oT_h' = [V_h | 1]^T @ E  => [65, i] (row 64 = softmax denom),
              oT_h = oT_h[0:64] * (1/denom) broadcast (GPSIMD bcast + DVE)
  out_partial = oT^T @ Wo_slice^T  [T, 1024]
All matmuls run in float32r (fp32 storage, full-rate PE mode, ~1e-5 rel err).
"""

import os
import sys

import numpy as np

if "/opt/trn_rl_repo" not in sys.path and os.path.isdir("/opt/trn_rl_repo"):
    sys.path.insert(0, "/opt/trn_rl_repo")

import concourse.tile as tile  # noqa: E402
from concourse import bacc, mybir  # noqa: E402
from concourse.bass_utils import run_bass_kernel_spmd  # noqa: E402

F32 = mybir.dt.float32
F32R = mybir.dt.float32r
AF = mybir.ActivationFunctionType
OP = mybir.AluOpType

B, T, D, H, DH = 4, 2048, 1024, 16, 64
HL = H // 2          # heads per core
E = HL * DH          # 512: per-core q (or k, or v) feature width
KO = D // 128        # 8 contraction chunks for the projections
P = 128
SCALE = float(D) ** -0.5  # note: dim**-0.5, faithful to the reference

_cache = {}
last_results = None


def _emit(ctx, tc, nc, xT, wq, wk, wv, wo, out, t, dbg=None):
    TB = t // 512        # moving-dim blocks for projections
    TC = t // 128        # t chunks (also j chunks)
    JC = t // 128
    IBN = t // 512       # query i-blocks
    GN = JC // 4         # groups of 4 j-chunks per i-block

    xT3 = xT.rearrange("(ko p) t -> p ko t", p=P)
    wq3 = wq.rearrange("(ko p) e -> p ko e", p=P)
    wk3 = wk.rearrange("(ko p) e -> p ko e", p=P)
    wv3 = wv.rearrange("(ko p) e -> p ko e", p=P)
    wo3 = wo.rearrange("(c p) d -> p c d", p=P)

    persist = ctx.enter_context(tc.tile_pool(name="persist", bufs=1))
    qkT = persist.tile([P, 8, t], F32R, tag="qkT")    # outer 0-3: q pairs, 4-7: k pairs
    vsb = persist.tile([P, JC, HL, DH + 1], F32R, tag="v")
    oT_all = persist.tile([P, 4, t], F32R, tag="oT")
    ones1 = persist.tile([P, 1], F32, tag="ones")
    nc.vector.memset(ones1[:], 1.0)
    nc.vector.tensor_copy(vsb[:, :, :, DH], ones1.to_broadcast((P, JC, HL)))

    # ---- phase 1a: V projection (x pass 1) ----
    with tc.tile_pool(name="wv_p", bufs=1) as wvp, \
         tc.tile_pool(name="xt1", bufs=2) as xtp1, \
         tc.tile_pool(name="ps1", bufs=4, space="PSUM") as ps1:
        wv_sb = wvp.tile([P, KO, E], F32R, tag="wv")
        nc.sync.dma_start(wv_sb[:], wv3)
        for tb in range(TB):
            xt = xtp1.tile([P, KO, 512], F32R, tag="xt")
            nc.sync.dma_start(xt[:], xT3[:, :, tb * 512:(tb + 1) * 512])
            for sub in range(4):
                tci = tb * 4 + sub
                ps = ps1.tile([P, 512], F32, tag="vps")
                for ko in range(KO):
                    nc.tensor.matmul(ps[:], xt[:, ko, sub * 128:(sub + 1) * 128],
                                     wv_sb[:, ko, :],
                                     start=(ko == 0), stop=(ko == KO - 1))
                nc.vector.tensor_copy(vsb[:, tci, :, 0:DH],
                                      ps.rearrange("p (h d) -> p h d", d=DH))

    # ---- phase 1b: Q and K projections (x pass 2) ----
    with tc.tile_pool(name="wqk_p", bufs=1) as wqkp, \
         tc.tile_pool(name="xt2", bufs=2) as xtp2, \
         tc.tile_pool(name="ps2", bufs=4, space="PSUM") as ps2:
        wq_sb = wqkp.tile([P, KO, E], F32R, tag="wq")
        wk_sb = wqkp.tile([P, KO, E], F32R, tag="wk")
        nc.sync.dma_start(wq_sb[:], wq3)
        nc.sync.dma_start(wk_sb[:], wk3)
        for tb in range(TB):
            xt = xtp2.tile([P, KO, 512], F32R, tag="xt")
            nc.sync.dma_start(xt[:], xT3[:, :, tb * 512:(tb + 1) * 512])
            for eo in range(8):
                wsb = wq_sb if eo < 4 else wk_sb
                ee = (eo % 4) * 128
                ps = ps2.tile([P, 512], F32, tag="qkps")
                for ko in range(KO):
                    nc.tensor.matmul(ps[:], wsb[:, ko, ee:ee + 128], xt[:, ko, :],
                                     start=(ko == 0), stop=(ko == KO - 1))
                nc.vector.tensor_copy(qkT[:, eo, tb * 512:(tb + 1) * 512], ps[:])

    # ---- phase 2: attention ----
    # Head pairs: head 2pr lives at partitions 0-63 ("A"), 2pr+1 at 64-127
    # ("B") — their QK matmuls run concurrently in the two row-halves of the
    # PE array.  Work is a stream of units (ib, head, jc); 3 units share one
    # 3-bank PSUM slot, one 1536-wide exp op each.  AV consumption lags one
    # slot behind QK/exp so the PE never waits on the ScalarE (which would
    # re-throttle the HAM clock); AV accumulates a whole i-block per head in
    # a dedicated PSUM bank, drained once per (ib, head).
    with tc.tile_pool(name="attn", bufs=1) as asb, \
         tc.tile_pool(name="expp", bufs=2) as expp, \
         tc.tile_pool(name="bc", bufs=1) as bcp, \
         tc.tile_pool(name="qkr", bufs=2, space="PSUM") as qkr, \
         tc.tile_pool(name="avp", bufs=1, space="PSUM") as avp:
        for pr in range(4):
            q_ = [qkT[0:64, pr, :], qkT[64:128, pr, :]]
            k_ = [qkT[0:64, 4 + pr, :], qkT[64:128, 4 + pr, :]]
            oacc = [asb.tile([65, t], F32, tag="oacc_a", name="oacc_a"),
                    asb.tile([65, t], F32, tag="oacc_b", name="oacc_b")]
            units = [(ib, hb, jc)
                     for ib in range(IBN) for jc in range(JC) for hb in range(2)]
            av_tiles = [None, None]

            def flush_av(prev, pr=pr, oacc=oacc, av_tiles=av_tiles):
                es, us = prev
                for idx, (ib, hb, jc) in enumerate(us):
                    if jc == 0:
                        av_tiles[hb] = avp.tile([65, 512], F32, tag=f"av{hb}",
                                                name=f"av{hb}")
                    nc.tensor.matmul(av_tiles[hb][:], vsb[:, jc, 2 * pr + hb, :],
                                     es[:, idx, :],
                                     start=(jc == 0), stop=(jc == JC - 1))
                    if jc == JC - 1:
                        nc.vector.tensor_copy(
                            oacc[hb][:, ib * 512:ib * 512 + 512], av_tiles[hb][:])

            pending = None  # (expS_tile, units_in_slot)
            for s0 in range(0, len(units), 3):
                us = units[s0:s0 + 3]
                ps = qkr.tile([P, 3, 512], F32, tag="qk", name="qk")
                for idx, (ib, hb, jc) in enumerate(us):
                    nc.tensor.matmul(ps[:, idx, :],
                                     k_[hb][:, jc * 128:(jc + 1) * 128],
                                     q_[hb][:, ib * 512:ib * 512 + 512],
                                     start=True, stop=True)
                es = expp.tile([P, 3, 512], F32R, tag="es", name="es")
                nc.scalar.activation(es[:, 0:len(us), :], ps[:, 0:len(us), :],
                                     AF.Exp, scale=SCALE)
                if pending is not None:
                    flush_av(pending)
                pending = (es, us)
            if pending is not None:
                flush_av(pending)
            # normalize: oT = oacc[0:64] * (1 / denom), denom = row 64.
            # The denom row sits at partition 64; engines can't shift
            # partitions, so DMA it to a partition-0 tile first.  The B half's
            # product must land at partitions 64-127 of oT_all — also a
            # partition shift, done with an SBUF->SBUF DMA of the product.
            rec = asb.tile([1, t], F32, tag="rec")
            nc.sync.dma_start(rec[:], oacc[0][64:65, :])
            nc.vector.reciprocal(rec[:], rec[:])
            ba = bcp.tile([64, t], F32, tag="bc", name="ba")
            nc.gpsimd.partition_broadcast(ba[:], rec[:])
            nc.vector.tensor_tensor(oT_all[0:64, pr, :], oacc[0][0:64, :], ba[:], OP.mult)
            rec2 = asb.tile([1, t], F32, tag="rec2")
            nc.sync.dma_start(rec2[:], oacc[1][64:65, :])
            nc.vector.reciprocal(rec2[:], rec2[:])
            bb = bcp.tile([64, t], F32, tag="bc2", name="bb")
            nc.gpsimd.partition_broadcast(bb[:], rec2[:])
            tmpb = bcp.tile([64, t], F32R, tag="tmpb")
            nc.vector.tensor_tensor(tmpb[:], oacc[1][0:64, :], bb[:], OP.mult)
            nc.sync.dma_start(oT_all[64:128, pr, :], tmpb[:])

    if dbg is not None:
        nc.sync.dma_start(dbg["qkT"], qkT[:].bitcast(F32))
        nc.sync.dma_start(dbg["vsb"], vsb[:].bitcast(F32))
        nc.sync.dma_start(dbg["oT"], oT_all[:].bitcast(F32))

    # ---- phase 3: output projection (partial over this core's heads) ----
    with tc.tile_pool(name="wo_p", bufs=1) as wop, \
         tc.tile_pool(name="op_ps", bufs=4, space="PSUM") as opps, \
         tc.tile_pool(name="osb", bufs=4) as osbp:
        wo_sb = wop.tile([P, 4, D], F32R, tag="wo")
        nc.sync.dma_start(wo_sb[:], wo3)
        for tci in range(TC):
            for db in range(D // 512):
                ps = opps.tile([P, 512], F32, tag="op")
                for c in range(4):
                    nc.tensor.matmul(ps[:], oT_all[:, c, tci * 128:(tci + 1) * 128],
                                     wo_sb[:, c, db * 512:(db + 1) * 512],
                                     start=(c == 0), stop=(c == 3))
                ot = osbp.tile([P, 512], F32, tag="ot")
                nc.vector.tensor_copy(ot[:], ps[:])
                nc.sync.dma_start(out[tci * 128:(tci + 1) * 128, db * 512:(db + 1) * 512],
                                  ot[:])


def _build(t, debug_outs=False):
    from contextlib import ExitStack

    nc = bacc.Bacc("TRN2", target_bir_lowering=False, debug=False, num_devices=8)
    xT = nc.dram_tensor("xT", [D, t], F32R, kind="ExternalInput").ap()
    wq = nc.dram_tensor("wq", [D, E], F32R, kind="ExternalInput").ap()
    wk = nc.dram_tensor("wk", [D, E], F32R, kind="ExternalInput").ap()
    wv = nc.dram_tensor("wv", [D, E], F32R, kind="ExternalInput").ap()
    wo = nc.dram_tensor("wo", [E, D], F32R, kind="ExternalInput").ap()
    out = nc.dram_tensor("out", [t, D], F32, kind="ExternalOutput").ap()
    dbg = None
    if debug_outs:
        JC = t // 128
        dbg = {
            "qkT": nc.dram_tensor("dbg_qkT", [P, 8, t], F32, kind="ExternalOutput").ap(),
            "vsb": nc.dram_tensor("dbg_vsb", [P, JC, HL, DH + 1], F32, kind="ExternalOutput").ap(),
            "oT": nc.dram_tensor("dbg_oT", [P, 4, t], F32, kind="ExternalOutput").ap(),
        }
    with tile.TileContext(nc) as tc:
        with ExitStack() as ctx:
            _emit(ctx, tc, nc, xT, wq, wk, wv, wo, out, t, dbg)
    nc.compile()
    return nc


def get_compiled(t=T, debug_outs=False):
    key = (t, debug_outs)
    if key not in _cache:
        _cache[key] = _build(t, debug_outs)
    return _cache[key]


def shard_inputs(x, w_qkv, t=T):
    """Per-core input maps (weights reordered head-major, x transposed)."""
    d_idx = np.arange(DH)
    maps = []
    for c in range(8):
        b = c // 2
        heads = np.arange((c % 2) * HL, (c % 2) * HL + HL)
        rows_q = (heads[:, None] + d_idx[None, :] * (3 * H)).reshape(-1)
        rows_k = (heads[:, None] + H + d_idx[None, :] * (3 * H)).reshape(-1)
        rows_v = (heads[:, None] + 2 * H + d_idx[None, :] * (3 * H)).reshape(-1)
        maps.append({
            "xT": np.ascontiguousarray(x[b][:t].T),
            "wq": np.ascontiguousarray(w_qkv[rows_q].T),
            "wk": np.ascontiguousarray(w_qkv[rows_k].T),
            "wv": np.ascontiguousarray(w_qkv[rows_v].T),
            "wo": None,  # filled below
        })
    return maps


def kernel(x, w_qkv, w_out, b_out):
    x = np.asarray(x, dtype=np.float32)
    w_qkv = np.asarray(w_qkv, dtype=np.float32)
    w_out = np.asarray(w_out, dtype=np.float32)
    b_out = np.asarray(b_out, dtype=np.float32)

    nc = get_compiled(T)
    d_idx = np.arange(DH)
    in_maps = shard_inputs(x, w_qkv, T)
    for c in range(8):
        heads = np.arange((c % 2) * HL, (c % 2) * HL + HL)
        cols_o = (heads[:, None] * DH + d_idx[None, :]).reshape(-1)
        in_maps[c]["wo"] = np.ascontiguousarray(w_out[:, cols_o].T)

    res = run_bass_kernel_spmd(nc, in_maps, core_ids=list(range(8)))
    global last_results
    last_results = res

    out = np.empty((B, T, D), dtype=np.float32)
    for b in range(B):
        out[b] = res.results[2 * b]["out"] + res.results[2 * b + 1]["out"]
    out += b_out
    return out.reshape(B, T, D)


# revision 29
# speedup vs baseline: 1.8389x; 1.1948x over previous
"""Multi-head attention (B=4, T=2048, D=1024, H=16, DH=64) on 8 Trainium2 NeuronCores.

Sharding (data + tensor parallel, no collectives):
  core c owns batch b = c//2 and heads [(c%2)*8, (c%2)*8 + 8).
  Each core computes q/k/v projections for its batch over its 8 heads, the
  full attention for those (batch, head) slabs, and a partial output
  projection over its heads' columns of w_out.  The host sums the two
  partial outputs per batch (the only cross-core reduction).

Device algorithm (per core), everything feature-major ("transposed") so the
contraction dim always lands on SBUF partitions:
  qT = Wq @ x^T            [512, T]   (rows = h_local*64 + d)
  kT = Wk @ x^T            [512, T]
  V  = x @ Wv^T            [T, 512]   (+ a ones column per head => denominator)
  per head h: ST = kT_h^T-contraction  => scores [j, i] in PSUM,
              E = exp(ST/32) (ScalarE, fused scale),
              oT_h' = [V_h | 1]^T @ E  => [65, i] (row 64 = softmax denom),
              oT_h = oT_h[0:64] * (1/denom) broadcast (GPSIMD bcast + DVE)
  out_partial = oT^T @ Wo_slice^T  [T, 1024]
All matmuls run in float32r (fp32 storage, full-rate PE mode, ~1e-5 rel err).
"""

import os
import sys

import numpy as np

if "/opt/trn_rl_repo" not in sys.path and os.path.isdir("/opt/trn_rl_repo"):
    sys.path.insert(0, "/opt/trn_rl_repo")

import concourse.bass as bass  # noqa: E402
import concourse.tile as tile  # noqa: E402
from concourse import bacc, mybir  # noqa: E402
from concourse.bass_utils import run_bass_kernel_spmd  # noqa: E402


def bass_AP_bcast(tile_ap, parts, free):
    """Partition-broadcast AP over a DRAM [1, free] tile (step-0 partition)."""
    a = tile_ap[:]
    return bass.AP(tensor=a.tensor, offset=a.offset,
                   ap=[[0, parts]] + [list(x) for x in a.ap[1:]])

F32 = mybir.dt.float32
F32R = mybir.dt.float32r
AF = mybir.ActivationFunctionType
OP = mybir.AluOpType

B, T, D, H, DH = 4, 2048, 1024, 16, 64
HL = H // 2          # heads per core
E = HL * DH          # 512: per-core q (or k, or v) feature width
KO = D // 128        # 8 contraction chunks for the projections
P = 128
SCALE = float(D) ** -0.5  # note: dim**-0.5, faithful to the reference

_cache = {}
last_results = None


def _emit(ctx, tc, nc, xT, wq, wk, wv, wo, out, t, dbg=None):
    TB = t // 512        # moving-dim blocks for projections
    TC = t // 128        # t chunks (also j chunks)
    JC = t // 128
    IBN = t // 512       # query i-blocks
    GN = JC // 4         # groups of 4 j-chunks per i-block

    xT3 = xT.rearrange("(ko p) t -> p ko t", p=P)
    wq3 = wq.rearrange("(ko p) e -> p ko e", p=P)
    wk3 = wk.rearrange("(ko p) e -> p ko e", p=P)
    wv3 = wv.rearrange("(ko p) e -> p ko e", p=P)
    wo3 = wo.rearrange("(c p) d -> p c d", p=P)

    persist = ctx.enter_context(tc.tile_pool(name="persist", bufs=1))
    qkT = persist.tile([P, 8, t], F32R, tag="qkT")    # outer 0-3: q pairs, 4-7: k pairs
    vsb = persist.tile([P, JC, HL, DH + 1], F32R, tag="v")
    ones1 = persist.tile([P, 1], F32, tag="ones")
    nc.vector.memset(ones1[:], 1.0)
    nc.vector.tensor_copy(vsb[:, :, :, DH], ones1.to_broadcast((P, JC, HL)))

    # ---- phase 1: V, Q, K projections in one pass over x ----
    with tc.tile_pool(name="w_p", bufs=1) as wp, \
         tc.tile_pool(name="xt1", bufs=2) as xtp1, \
         tc.tile_pool(name="ps1", bufs=4, space="PSUM") as ps1:
        wv_sb = wp.tile([P, KO, E], F32R, tag="wv")
        wq_sb = wp.tile([P, KO, E], F32R, tag="wq")
        wk_sb = wp.tile([P, KO, E], F32R, tag="wk")
        xt0 = xtp1.tile([P, KO, 512], F32R, tag="xt", name="xt0")
        nc.sync.dma_start(xt0[:], xT3[:, :, 0:512])
        for ko in range(KO):
            nc.sync.dma_start(wv_sb[:, ko, :], wv3[:, ko, :])
        for ko in range(KO):
            nc.sync.dma_start(wq_sb[:, ko, :], wq3[:, ko, :])
            nc.sync.dma_start(wk_sb[:, ko, :], wk3[:, ko, :])
        for tb in range(TB):
            if tb == 0:
                xt = xt0
            else:
                xt = xtp1.tile([P, KO, 512], F32R, tag="xt", name="xt")
                nc.sync.dma_start(xt[:], xT3[:, :, tb * 512:(tb + 1) * 512])
            for sub in range(4):
                tci = tb * 4 + sub
                ps = ps1.tile([P, 512], F32, tag="vps")
                for ko in range(KO):
                    nc.tensor.matmul(ps[:], xt[:, ko, sub * 128:(sub + 1) * 128],
                                     wv_sb[:, ko, :],
                                     start=(ko == 0), stop=(ko == KO - 1))
                nc.vector.tensor_copy(vsb[:, tci, :, 0:DH],
                                      ps.rearrange("p (h d) -> p h d", d=DH))
            for eo in range(8):
                wsb = wq_sb if eo < 4 else wk_sb
                ee = (eo % 4) * 128
                ps = ps1.tile([P, 512], F32, tag="qkps")
                for ko in range(KO):
                    nc.tensor.matmul(ps[:], wsb[:, ko, ee:ee + 128], xt[:, ko, :],
                                     start=(ko == 0), stop=(ko == KO - 1))
                nc.vector.tensor_copy(qkT[:, eo, tb * 512:(tb + 1) * 512], ps[:])

    # ---- phase 2: attention ----
    # Head pairs: head 2pr lives at partitions 0-63 ("A"), 2pr+1 at 64-127
    # ("B") — their QK matmuls run concurrently in the two row-halves of the
    # PE array.  Work is a stream of units (ib, head, jc); 3 units share one
    # 3-bank PSUM slot, one 1536-wide exp op each.  AV consumption lags one
    # slot behind QK/exp so the PE never waits on the ScalarE (which would
    # re-throttle the HAM clock); AV accumulates a whole i-block per head in
    # a dedicated PSUM bank, drained once per (ib, head).
    with tc.tile_pool(name="attn", bufs=1) as asb, \
         tc.tile_pool(name="expp", bufs=2) as expp, \
         tc.tile_pool(name="bc", bufs=1) as bcp, \
         tc.tile_pool(name="wo_p", bufs=1) as wop, \
         tc.tile_pool(name="dram", bufs=2, space="DRAM") as dramp:
        wo_sb = wop.tile([P, 4, D], F32R, tag="wo")
        nc.sync.dma_start(wo_sb[:], wo3)
        oT_all = asb.tile([P, 4, t], F32R, tag="oT")

        def normalize(pr, oacc):
            # oT = oacc[0:64] * (1 / denom), denom = row 64.  The denom row
            # sits on ONE partition; a [1, t] DVE reciprocal would run on a
            # single lane (~13 us).  Bounce it through DRAM to reshape to
            # [128, t/128] (reciprocal on 128 lanes), back to DRAM, then
            # DMA-broadcast from DRAM across 64 partitions.  The B-half
            # product needs a partition shift to rows 64-127: SBUF->SBUF DMA.
            for hb in range(2):
                dd1 = dramp.tile([1, t], F32, tag="dd1", name="dd1")
                nc.sync.dma_start(dd1[:], oacc[hb][64:65, :])
                den = asb.tile([P, t // P], F32, tag="den")
                nc.sync.dma_start(den[:], dd1.rearrange("o (p f) -> (o p) f", p=P))
                nc.vector.reciprocal(den[:], den[:])
                dd2 = dramp.tile([1, t], F32, tag="dd2", name="dd2")
                nc.sync.dma_start(dd2.rearrange("o (p f) -> (o p) f", p=P), den[:])
                bc = bcp.tile([64, t], F32, tag="bc", name="bc")
                nc.sync.dma_start(bc[:], bass_AP_bcast(dd2, 64, t))
                if hb == 0:
                    nc.vector.tensor_tensor(oT_all[0:64, pr, :], oacc[0][0:64, :],
                                            bc[:], OP.mult)
                else:
                    tmpb = bcp.tile([64, t], F32R, tag="tmpb")
                    nc.vector.tensor_tensor(tmpb[:], oacc[1][0:64, :], bc[:], OP.mult)
                    nc.sync.dma_start(oT_all[64:128, pr, :], tmpb[:])

        oacc3 = None
        with tc.tile_pool(name="qkr", bufs=2, space="PSUM") as qkr, \
             tc.tile_pool(name="avp", bufs=1, space="PSUM") as avp:
            for pr in range(4):
                q_ = [qkT[0:64, pr, :], qkT[64:128, pr, :]]
                k_ = [qkT[0:64, 4 + pr, :], qkT[64:128, 4 + pr, :]]
                oacc = [asb.tile([65, t], F32, tag="oacc_a", name="oacc_a"),
                        asb.tile([65, t], F32, tag="oacc_b", name="oacc_b")]
                units = [(ib, hb, jc)
                         for ib in range(IBN) for jc in range(JC) for hb in range(2)]
                av_tiles = [None, None]

                def flush_av(prev, pr=pr, oacc=oacc, av_tiles=av_tiles):
                    es, us = prev
                    for idx, (ib, hb, jc) in enumerate(us):
                        if jc == 0:
                            av_tiles[hb] = avp.tile([65, 512], F32, tag=f"av{hb}",
                                                    name=f"av{hb}")
                        nc.tensor.matmul(av_tiles[hb][:], vsb[:, jc, 2 * pr + hb, :],
                                         es[:, idx, :],
                                         start=(jc == 0), stop=(jc == JC - 1))
                        if jc == JC - 1:
                            nc.vector.tensor_copy(
                                oacc[hb][:, ib * 512:ib * 512 + 512], av_tiles[hb][:])

                pending = None  # (expS_tile, units_in_slot)
                for s0 in range(0, len(units), 3):
                    us = units[s0:s0 + 3]
                    ps = qkr.tile([P, 3, 512], F32, tag="qk", name="qk")
                    for idx, (ib, hb, jc) in enumerate(us):
                        nc.tensor.matmul(ps[:, idx, :],
                                         k_[hb][:, jc * 128:(jc + 1) * 128],
                                         q_[hb][:, ib * 512:ib * 512 + 512],
                                         start=True, stop=True)
                    es = expp.tile([P, 3, 512], F32R, tag="es", name="es")
                    nc.scalar.activation(
                        es[:, 0:len(us), :].rearrange("p a b -> p (a b)"),
                        ps[:, 0:len(us), :].rearrange("p a b -> p (a b)"),
                        AF.Exp, scale=SCALE)
                    if pending is not None:
                        flush_av(pending)
                    pending = (es, us)
                if pending is not None:
                    flush_av(pending)
                if pr < 3:
                    normalize(pr, oacc)
                else:
                    oacc3 = oacc

        if dbg is not None:
            nc.sync.dma_start(dbg["vsb"], vsb[:].bitcast(F32))

        # ---- phase 3: output projection (partial over this core's heads) ----
        # Start the pair-3 normalize (DMA/DVE) immediately; meanwhile the PE
        # runs the first 8 blocks' c=0..2 partial accumulations (they only
        # need pairs 0-2), so it stays busy/warm.  Those groups finish with
        # c=3 once the normalize lands.
        with tc.tile_pool(name="op_ps", bufs=8, space="PSUM") as opps, \
             tc.tile_pool(name="osb", bufs=4) as osbp:
            normalize(3, oacc3)

            def op_block(tci, db, ps, c_lo, c_hi):
                for c in range(c_lo, c_hi):
                    nc.tensor.matmul(ps[:], oT_all[:, c, tci * 128:(tci + 1) * 128],
                                     wo_sb[:, c, db * 512:(db + 1) * 512],
                                     start=(c == 0), stop=(c == 3))

            def op_finish(tci, db, ps):
                ot = osbp.tile([P, 512], F32, tag="ot", name="ot")
                nc.vector.tensor_copy(ot[:], ps[:])
                nc.sync.dma_start(out[tci * 128:(tci + 1) * 128,
                                      db * 512:(db + 1) * 512], ot[:])

            blocks = [(tci, db) for tci in range(TC) for db in range(D // 512)]
            early = []
            for tci, db in blocks[:8]:
                ps = opps.tile([P, 512], F32, tag="op", name="op")
                op_block(tci, db, ps, 0, 3)
                early.append((tci, db, ps))
            for tci, db, ps in early:
                op_block(tci, db, ps, 3, 4)
                op_finish(tci, db, ps)
            for tci, db in blocks[8:]:
                ps = opps.tile([P, 512], F32, tag="op", name="op")
                op_block(tci, db, ps, 0, 4)
                op_finish(tci, db, ps)

        if dbg is not None:
            nc.sync.dma_start(dbg["qkT"], qkT[:].bitcast(F32))
            nc.sync.dma_start(dbg["oT"], oT_all[:].bitcast(F32))


def _build(t, debug_outs=False):
    from contextlib import ExitStack

    nc = bacc.Bacc("TRN2", target_bir_lowering=False, debug=False, num_devices=8)
    xT = nc.dram_tensor("xT", [D, t], F32R, kind="ExternalInput").ap()
    wq = nc.dram_tensor("wq", [D, E], F32R, kind="ExternalInput").ap()
    wk = nc.dram_tensor("wk", [D, E], F32R, kind="ExternalInput").ap()
    wv = nc.dram_tensor("wv", [D, E], F32R, kind="ExternalInput").ap()
    wo = nc.dram_tensor("wo", [E, D], F32R, kind="ExternalInput").ap()
    out = nc.dram_tensor("out", [t, D], F32, kind="ExternalOutput").ap()
    dbg = None
    if debug_outs:
        JC = t // 128
        dbg = {
            "qkT": nc.dram_tensor("dbg_qkT", [P, 8, t], F32, kind="ExternalOutput").ap(),
            "vsb": nc.dram_tensor("dbg_vsb", [P, JC, HL, DH + 1], F32, kind="ExternalOutput").ap(),
            "oT": nc.dram_tensor("dbg_oT", [P, 4, t], F32, kind="ExternalOutput").ap(),
        }
    with tile.TileContext(nc) as tc:
        with ExitStack() as ctx:
            _emit(ctx, tc, nc, xT, wq, wk, wv, wo, out, t, dbg)
    nc.compile()
    return nc


def get_compiled(t=T, debug_outs=False):
    key = (t, debug_outs)
    if key not in _cache:
        _cache[key] = _build(t, debug_outs)
    return _cache[key]


def shard_inputs(x, w_qkv, t=T):
    """Per-core input maps (weights reordered head-major, x transposed)."""
    d_idx = np.arange(DH)
    maps = []
    for c in range(8):
        b = c // 2
        heads = np.arange((c % 2) * HL, (c % 2) * HL + HL)
        rows_q = (heads[:, None] + d_idx[None, :] * (3 * H)).reshape(-1)
        rows_k = (heads[:, None] + H + d_idx[None, :] * (3 * H)).reshape(-1)
        rows_v = (heads[:, None] + 2 * H + d_idx[None, :] * (3 * H)).reshape(-1)
        maps.append({
            "xT": np.ascontiguousarray(x[b][:t].T),
            "wq": np.ascontiguousarray(w_qkv[rows_q].T),
            "wk": np.ascontiguousarray(w_qkv[rows_k].T),
            "wv": np.ascontiguousarray(w_qkv[rows_v].T),
            "wo": None,  # filled below
        })
    return maps


def kernel(x, w_qkv, w_out, b_out):
    x = np.asarray(x, dtype=np.float32)
    w_qkv = np.asarray(w_qkv, dtype=np.float32)
    w_out = np.asarray(w_out, dtype=np.float32)
    b_out = np.asarray(b_out, dtype=np.float32)

    nc = get_compiled(T)
    d_idx = np.arange(DH)
    in_maps = shard_inputs(x, w_qkv, T)
    for c in range(8):
        heads = np.arange((c % 2) * HL, (c % 2) * HL + HL)
        cols_o = (heads[:, None] * DH + d_idx[None, :]).reshape(-1)
        in_maps[c]["wo"] = np.ascontiguousarray(w_out[:, cols_o].T)

    res = run_bass_kernel_spmd(nc, in_maps, core_ids=list(range(8)))
    global last_results
    last_results = res

    out = np.empty((B, T, D), dtype=np.float32)
    for b in range(B):
        out[b] = res.results[2 * b]["out"] + res.results[2 * b + 1]["out"]
    out += b_out
    return out.reshape(B, T, D)


# revision 30
# speedup vs baseline: 1.8564x; 1.0095x over previous
"""Multi-head attention (B=4, T=2048, D=1024, H=16, DH=64) on 8 Trainium2 NeuronCores.

Sharding (data + tensor parallel, no collectives):
  core c owns batch b = c//2 and heads [(c%2)*8, (c%2)*8 + 8).
  Each core computes q/k/v projections for its batch over its 8 heads, the
  full attention for those (batch, head) slabs, and a partial output
  projection over its heads' columns of w_out.  The host sums the two
  partial outputs per batch (the only cross-core reduction).

Device algorithm (per core), everything feature-major ("transposed") so the
contraction dim always lands on SBUF partitions:
  qT = Wq @ x^T            [512, T]   (rows = h_local*64 + d)
  kT = Wk @ x^T            [512, T]
  V  = x @ Wv^T            [T, 512]   (+ a ones column per head => denominator)
  per head h: ST = kT_h^T-contraction  => scores [j, i] in PSUM,
              E = exp(ST/32) (ScalarE, fused scale),
              oT_h' = [V_h | 1]^T @ E  => [65, i] (row 64 = softmax denom),
              oT_h = oT_h[0:64] * (1/denom) (reciprocal reshaped to 128 lanes
              via a DRAM bounce, then DMA-broadcast across partitions)
  out_partial = oT^T @ Wo_slice^T  [T, 1024]
All matmuls run in float32r (fp32 storage, full-rate PE mode, ~1e-5 rel err).
The attention inner loop is a lag-1 software pipeline, QK(s) -> exp(s) ->
AV(s-1), so the TensorE never stalls on ScalarE (which would re-throttle the
PE HAM clock to 1.2 GHz); AV accumulates each i-block in dedicated PSUM banks.
"""

import os
import sys

import numpy as np

if "/opt/trn_rl_repo" not in sys.path and os.path.isdir("/opt/trn_rl_repo"):
    sys.path.insert(0, "/opt/trn_rl_repo")

import concourse.bass as bass  # noqa: E402
import concourse.tile as tile  # noqa: E402
from concourse import bacc, mybir  # noqa: E402
from concourse.bass_utils import run_bass_kernel_spmd  # noqa: E402


def bass_AP_bcast(tile_ap, parts, free):
    """Partition-broadcast AP over a DRAM [1, free] tile (step-0 partition)."""
    a = tile_ap[:]
    return bass.AP(tensor=a.tensor, offset=a.offset,
                   ap=[[0, parts]] + [list(x) for x in a.ap[1:]])

F32 = mybir.dt.float32
F32R = mybir.dt.float32r
AF = mybir.ActivationFunctionType
OP = mybir.AluOpType

B, T, D, H, DH = 4, 2048, 1024, 16, 64
HL = H // 2          # heads per core
E = HL * DH          # 512: per-core q (or k, or v) feature width
KO = D // 128        # 8 contraction chunks for the projections
P = 128
SCALE = float(D) ** -0.5  # note: dim**-0.5, faithful to the reference

_cache = {}
last_results = None


def _emit(ctx, tc, nc, xT, wq, wk, wv, wo, out, t, dbg=None):
    TB = t // 512        # moving-dim blocks for projections
    TC = t // 128        # t chunks (also j chunks)
    JC = t // 128
    IBN = t // 512       # query i-blocks
    GN = JC // 4         # groups of 4 j-chunks per i-block

    xT3 = xT.rearrange("(ko p) t -> p ko t", p=P)
    wq3 = wq.rearrange("(ko p) e -> p ko e", p=P)
    wk3 = wk.rearrange("(ko p) e -> p ko e", p=P)
    wv3 = wv.rearrange("(ko p) e -> p ko e", p=P)
    wo3 = wo.rearrange("(c p) d -> p c d", p=P)

    persist = ctx.enter_context(tc.tile_pool(name="persist", bufs=1))
    qkT = persist.tile([P, 8, t], F32R, tag="qkT")    # outer 0-3: q pairs, 4-7: k pairs
    vsb = persist.tile([P, JC, HL, DH + 1], F32R, tag="v")
    ones1 = persist.tile([P, 1], F32, tag="ones")
    nc.vector.memset(ones1[:], 1.0)
    nc.vector.tensor_copy(vsb[:, :, :, DH], ones1.to_broadcast((P, JC, HL)))

    # ---- phase 1: V, Q, K projections in one pass over x ----
    with tc.tile_pool(name="w_p", bufs=1) as wp, \
         tc.tile_pool(name="xt1", bufs=2) as xtp1, \
         tc.tile_pool(name="ps1", bufs=4, space="PSUM") as ps1:
        wv_sb = wp.tile([P, KO, E], F32R, tag="wv")
        wq_sb = wp.tile([P, KO, E], F32R, tag="wq")
        wk_sb = wp.tile([P, KO, E], F32R, tag="wk")
        xt0 = xtp1.tile([P, KO, 512], F32R, tag="xt", name="xt0")
        nc.sync.dma_start(xt0[:], xT3[:, :, 0:512])
        for ko in range(KO):
            nc.sync.dma_start(wv_sb[:, ko, :], wv3[:, ko, :])
        for ko in range(KO):
            nc.sync.dma_start(wq_sb[:, ko, :], wq3[:, ko, :])
            nc.sync.dma_start(wk_sb[:, ko, :], wk3[:, ko, :])
        for tb in range(TB):
            if tb == 0:
                xt = xt0
            else:
                xt = xtp1.tile([P, KO, 512], F32R, tag="xt", name="xt")
                nc.sync.dma_start(xt[:], xT3[:, :, tb * 512:(tb + 1) * 512])
            for sub in range(4):
                tci = tb * 4 + sub
                ps = ps1.tile([P, 512], F32, tag="vps")
                for ko in range(KO):
                    nc.tensor.matmul(ps[:], xt[:, ko, sub * 128:(sub + 1) * 128],
                                     wv_sb[:, ko, :],
                                     start=(ko == 0), stop=(ko == KO - 1))
                nc.vector.tensor_copy(vsb[:, tci, :, 0:DH],
                                      ps.rearrange("p (h d) -> p h d", d=DH))
            for eo in range(8):
                wsb = wq_sb if eo < 4 else wk_sb
                ee = (eo % 4) * 128
                ps = ps1.tile([P, 512], F32, tag="qkps")
                for ko in range(KO):
                    nc.tensor.matmul(ps[:], wsb[:, ko, ee:ee + 128], xt[:, ko, :],
                                     start=(ko == 0), stop=(ko == KO - 1))
                nc.vector.tensor_copy(qkT[:, eo, tb * 512:(tb + 1) * 512], ps[:])

    # ---- phase 2: attention ----
    # Head pairs: head 2pr lives at partitions 0-63 ("A"), 2pr+1 at 64-127
    # ("B") — their QK matmuls run concurrently in the two row-halves of the
    # PE array.  Work is a stream of units (ib, head, jc); 3 units share one
    # 3-bank PSUM slot, one 1536-wide exp op each.  AV consumption lags one
    # slot behind QK/exp so the PE never waits on the ScalarE (which would
    # re-throttle the HAM clock); AV accumulates a whole i-block per head in
    # a dedicated PSUM bank, drained once per (ib, head).
    with tc.tile_pool(name="attn", bufs=1) as asb, \
         tc.tile_pool(name="expp", bufs=2) as expp, \
         tc.tile_pool(name="bc", bufs=1) as bcp, \
         tc.tile_pool(name="wo_p", bufs=1) as wop, \
         tc.tile_pool(name="dram", bufs=2, space="DRAM") as dramp:
        wo_sb = wop.tile([P, 4, D], F32R, tag="wo")
        nc.sync.dma_start(wo_sb[:], wo3)
        oT_all = asb.tile([P, 4, t], F32R, tag="oT")

        def normalize(pr, oacc):
            # oT = oacc[0:64] * (1 / denom), denom = row 64.  The denom row
            # sits on ONE partition; a [1, t] DVE reciprocal would run on a
            # single lane (~13 us).  Bounce it through DRAM to reshape to
            # [128, t/128] (reciprocal on 128 lanes), back to DRAM, then
            # DMA-broadcast from DRAM across 64 partitions.  The B-half
            # product needs a partition shift to rows 64-127: SBUF->SBUF DMA.
            for hb in range(2):
                dd1 = dramp.tile([1, t], F32, tag="dd1", name="dd1")
                nc.sync.dma_start(dd1[:], oacc[hb][64:65, :])
                den = asb.tile([P, t // P], F32, tag="den")
                nc.sync.dma_start(den[:], dd1.rearrange("o (p f) -> (o p) f", p=P))
                nc.vector.reciprocal(den[:], den[:])
                dd2 = dramp.tile([1, t], F32, tag="dd2", name="dd2")
                nc.sync.dma_start(dd2.rearrange("o (p f) -> (o p) f", p=P), den[:])
                bc = bcp.tile([64, t], F32, tag="bc", name="bc")
                nc.sync.dma_start(bc[:], bass_AP_bcast(dd2, 64, t))
                if hb == 0:
                    nc.vector.tensor_tensor(oT_all[0:64, pr, :], oacc[0][0:64, :],
                                            bc[:], OP.mult)
                else:
                    tmpb = bcp.tile([64, t], F32R, tag="tmpb")
                    nc.vector.tensor_tensor(tmpb[:], oacc[1][0:64, :], bc[:], OP.mult)
                    nc.sync.dma_start(oT_all[64:128, pr, :], tmpb[:])

        oacc3 = None
        with tc.tile_pool(name="qkr", bufs=2, space="PSUM") as qkr, \
             tc.tile_pool(name="avp", bufs=1, space="PSUM") as avp:
            for pr in range(4):
                q_ = [qkT[0:64, pr, :], qkT[64:128, pr, :]]
                k_ = [qkT[0:64, 4 + pr, :], qkT[64:128, 4 + pr, :]]
                oacc = [asb.tile([65, t], F32, tag="oacc_a", name="oacc_a"),
                        asb.tile([65, t], F32, tag="oacc_b", name="oacc_b")]
                units = [(ib, hb, jc)
                         for ib in range(IBN) for jc in range(JC) for hb in range(2)]
                av_tiles = [None, None]

                def flush_av(prev, pr=pr, oacc=oacc, av_tiles=av_tiles):
                    es, us = prev
                    for idx, (ib, hb, jc) in enumerate(us):
                        if jc == 0:
                            av_tiles[hb] = avp.tile([65, 512], F32, tag=f"av{hb}",
                                                    name=f"av{hb}")
                        nc.tensor.matmul(av_tiles[hb][:], vsb[:, jc, 2 * pr + hb, :],
                                         es[:, idx, :],
                                         start=(jc == 0), stop=(jc == JC - 1))
                        if jc == JC - 1:
                            nc.vector.tensor_copy(
                                oacc[hb][:, ib * 512:ib * 512 + 512], av_tiles[hb][:])

                pending = None  # (expS_tile, units_in_slot)
                for s0 in range(0, len(units), 3):
                    us = units[s0:s0 + 3]
                    ps = qkr.tile([P, 3, 512], F32, tag="qk", name="qk")
                    for idx, (ib, hb, jc) in enumerate(us):
                        nc.tensor.matmul(ps[:, idx, :],
                                         k_[hb][:, jc * 128:(jc + 1) * 128],
                                         q_[hb][:, ib * 512:ib * 512 + 512],
                                         start=True, stop=True)
                    es = expp.tile([P, 3, 512], F32R, tag="es", name="es")
                    nc.scalar.activation(
                        es[:, 0:len(us), :].rearrange("p a b -> p (a b)"),
                        ps[:, 0:len(us), :].rearrange("p a b -> p (a b)"),
                        AF.Exp, scale=SCALE)
                    if pending is not None:
                        flush_av(pending)
                    pending = (es, us)
                if pending is not None:
                    flush_av(pending)
                if pr < 3:
                    normalize(pr, oacc)
                else:
                    oacc3 = oacc

        if dbg is not None:
            nc.sync.dma_start(dbg["vsb"], vsb[:].bitcast(F32))

        # ---- phase 3: output projection (partial over this core's heads) ----
        # Start the pair-3 normalize (DMA/DVE) immediately; meanwhile the PE
        # runs the first 8 blocks' c=0..2 partial accumulations (they only
        # need pairs 0-2), so it stays busy/warm.  Those groups finish with
        # c=3 once the normalize lands.
        with tc.tile_pool(name="op_ps", bufs=8, space="PSUM") as opps, \
             tc.tile_pool(name="osb", bufs=4) as osbp:
            normalize(3, oacc3)

            def op_block(tci, db, ps, c_lo, c_hi):
                for c in range(c_lo, c_hi):
                    nc.tensor.matmul(ps[:], oT_all[:, c, tci * 128:(tci + 1) * 128],
                                     wo_sb[:, c, db * 512:(db + 1) * 512],
                                     start=(c == 0), stop=(c == 3))

            def op_finish(tci, db, ps):
                ot = osbp.tile([P, 512], F32, tag="ot", name="ot")
                nc.vector.tensor_copy(ot[:], ps[:])
                nc.sync.dma_start(out[tci * 128:(tci + 1) * 128,
                                      db * 512:(db + 1) * 512], ot[:])

            blocks = [(tci, db) for tci in range(TC) for db in range(D // 512)]
            early = []
            for tci, db in blocks[:8]:
                ps = opps.tile([P, 512], F32, tag="op", name="op")
                op_block(tci, db, ps, 0, 3)
                early.append((tci, db, ps))
            for tci, db, ps in early:
                op_block(tci, db, ps, 3, 4)
                op_finish(tci, db, ps)
            for tci, db in blocks[8:]:
                ps = opps.tile([P, 512], F32, tag="op", name="op")
                op_block(tci, db, ps, 0, 4)
                op_finish(tci, db, ps)

        if dbg is not None:
            nc.sync.dma_start(dbg["qkT"], qkT[:].bitcast(F32))
            nc.sync.dma_start(dbg["oT"], oT_all[:].bitcast(F32))


def _build(t, debug_outs=False):
    from contextlib import ExitStack

    nc = bacc.Bacc("TRN2", target_bir_lowering=False, debug=False, num_devices=8)
    xT = nc.dram_tensor("xT", [D, t], F32R, kind="ExternalInput").ap()
    wq = nc.dram_tensor("wq", [D, E], F32R, kind="ExternalInput").ap()
    wk = nc.dram_tensor("wk", [D, E], F32R, kind="ExternalInput").ap()
    wv = nc.dram_tensor("wv", [D, E], F32R, kind="ExternalInput").ap()
    wo = nc.dram_tensor("wo", [E, D], F32R, kind="ExternalInput").ap()
    out = nc.dram_tensor("out", [t, D], F32, kind="ExternalOutput").ap()
    dbg = None
    if debug_outs:
        JC = t // 128
        dbg = {
            "qkT": nc.dram_tensor("dbg_qkT", [P, 8, t], F32, kind="ExternalOutput").ap(),
            "vsb": nc.dram_tensor("dbg_vsb", [P, JC, HL, DH + 1], F32, kind="ExternalOutput").ap(),
            "oT": nc.dram_tensor("dbg_oT", [P, 4, t], F32, kind="ExternalOutput").ap(),
        }
    with tile.TileContext(nc) as tc:
        with ExitStack() as ctx:
            _emit(ctx, tc, nc, xT, wq, wk, wv, wo, out, t, dbg)
    nc.compile()
    return nc


def get_compiled(t=T, debug_outs=False):
    key = (t, debug_outs)
    if key not in _cache:
        _cache[key] = _build(t, debug_outs)
    return _cache[key]


def shard_inputs(x, w_qkv, t=T):
    """Per-core input maps (weights reordered head-major, x transposed)."""
    d_idx = np.arange(DH)
    maps = []
    for c in range(8):
        b = c // 2
        heads = np.arange((c % 2) * HL, (c % 2) * HL + HL)
        rows_q = (heads[:, None] + d_idx[None, :] * (3 * H)).reshape(-1)
        rows_k = (heads[:, None] + H + d_idx[None, :] * (3 * H)).reshape(-1)
        rows_v = (heads[:, None] + 2 * H + d_idx[None, :] * (3 * H)).reshape(-1)
        maps.append({
            "xT": np.ascontiguousarray(x[b][:t].T),
            "wq": np.ascontiguousarray(w_qkv[rows_q].T),
            "wk": np.ascontiguousarray(w_qkv[rows_k].T),
            "wv": np.ascontiguousarray(w_qkv[rows_v].T),
            "wo": None,  # filled below
        })
    return maps


def kernel(x, w_qkv, w_out, b_out):
    x = np.asarray(x, dtype=np.float32)
    w_qkv = np.asarray(w_qkv, dtype=np.float32)
    w_out = np.asarray(w_out, dtype=np.float32)
    b_out = np.asarray(b_out, dtype=np.float32)

    nc = get_compiled(T)
    d_idx = np.arange(DH)
    in_maps = shard_inputs(x, w_qkv, T)
    for c in range(8):
        heads = np.arange((c % 2) * HL, (c % 2) * HL + HL)
        cols_o = (heads[:, None] * DH + d_idx[None, :]).reshape(-1)
        in_maps[c]["wo"] = np.ascontiguousarray(w_out[:, cols_o].T)

    res = run_bass_kernel_spmd(nc, in_maps, core_ids=list(range(8)))
    global last_results
    last_results = res

    out = np.empty((B, T, D), dtype=np.float32)
    for b in range(B):
        out[b] = res.results[2 * b]["out"] + res.results[2 * b + 1]["out"]
    out += b_out
    return out.reshape(B, T, D)
